# revision 1
# baseline (speedup 1.0000x reference)
"""BinaryLayerWrapper (sync-BN + sign + binarized 3x3 conv) on 8 TRN2 cores.

Strategy (data-parallel, per sharding hint):
  - shard batch B=32 -> 4 images per core; conv weights replicated
  - phase A: stream x shard to SBUF (kept resident), per-channel partial
    sums sum(x), sum(x^2) over local batch+space; weight prep overlapped
    (alpha = mean|w|, sign(w), PE-transpose to [Cin,Cout] tiles)
  - tiny AllReduce (add) of [128,4] partial stats across 8 cores (sync-BN)
  - per-channel a = gamma*rsqrt(var+eps), b = beta - mean*a
  - phase C: xb = Sign(a*x+b) in fp8/bf16 written into zero-padded 58x58
    planes; 3x3 conv = 9 (fp8 DoubleRow) or 18 (bf16) accumulated matmuls
    per output tile on the PE array (N=464 = 8 output rows x 58 padded
    cols), then scale by alpha and DMA the valid interior out.

The conv math is exact: xb is +-1 (exact in fp8e4m3/bf16), sign(w) is
+-1, products accumulate in fp32 PSUM as small integers; alpha scaling
happens once at the end.

Instruction emission order is engine-FIFO-aware: engines execute their
queues in program order, so the sync-BN critical chain (stats ->
allreduce -> coefs -> first sign) is emitted before bulk weight-prep
work on the same engines.
"""

import os
from contextlib import ExitStack

import numpy as np

from concourse import bacc, bass, masks, mybir, tile
from concourse.bass_utils import run_bass_kernel_spmd

F32 = mybir.dt.float32
BF16 = mybir.dt.bfloat16
FP8 = mybir.dt.float8e4

# fp8e4m3 + DoubleRow (2 Cin chunks per matmul pass)
USE_FP8 = os.environ.get("USE_FP8", "1") == "1"

N_CORES = 8
B_LOC = 4          # images per core (32 / 8)
C = 256            # channels (in == out)
KC = 2             # 128-partition channel chunks
H = W = 56
PIX = H * W        # 3136
WP = W + 2         # 58 padded width
PLANE = WP * (H + 2)          # 58*58 = 3364
XBP_LEN = PLANE + 2           # +1 lead pad so all tap offsets are >= 0
PLANE_PAD = 3376              # XBP_LEN rounded to 16 (fp8 DoubleRow Ko step)
R = 8                         # output rows per matmul tile (N=464, 1 PSUM bank)
NF = R * WP                   # 464 matmul free dim
N_TOTAL = 32 * PIX            # full-batch elements per channel (sync-BN)


def build_program(num_devices: int = N_CORES, cc: bool = True,
                  stage: int = 3) -> bass.Bass:
    nc = bacc.Bacc("TRN2", target_bir_lowering=False, debug=False,
                   num_devices=num_devices)
    nc._use_cc = cc
    nc._cc_devices = num_devices
    nc._stage = stage

    x = nc.dram_tensor("x", [B_LOC, C, H, W], F32, kind="ExternalInput").ap()
    w = nc.dram_tensor("weight", [C, C, 3, 3], F32, kind="ExternalInput").ap()
    gamma = nc.dram_tensor("gamma", [C], F32, kind="ExternalInput").ap()
    beta = nc.dram_tensor("beta", [C], F32, kind="ExternalInput").ap()
    y = nc.dram_tensor("y", [B_LOC, C, H, W], F32, kind="ExternalOutput").ap()

    with tile.TileContext(nc) as tc:
        _body(tc, y, x, w, gamma, beta)
    nc.compile()
    return nc


def _body(tc: tile.TileContext, y, x, w, gamma, beta):
    nc = tc.nc
    add = mybir.AluOpType.add
    AF = mybir.ActivationFunctionType

    with (
        tc.tile_pool(name="singles", bufs=1) as singles,
        tc.tile_pool(name="wsbuf", bufs=1) as wspool,
        tc.tile_pool(name="xres", bufs=1) as xpool,
        tc.tile_pool(name="dram", bufs=1, space="DRAM") as dram,
    ):
        identity = singles.tile([128, 128], BF16, tag="identity")
        masks.make_identity(nc, identity[:])

        gb = singles.tile([128, 4], F32, tag="gb")  # cols: gamma k0,k1, beta k0,k1

        # per-(b,k,half) stat partials; cols indexed (k*B_LOC + b)*2 + h
        psum_parts = singles.tile([128, KC * B_LOC * 2], F32, tag="psum_parts")
        psq_parts = singles.tile([128, KC * B_LOC * 2], F32, tag="psq_parts")
        stats_local = singles.tile([128, 4], F32, tag="stats_local")
        gstats = singles.tile([128, 4], F32, tag="gstats")
        alpha = singles.tile([128, 2], F32, tag="alpha")        # per-o-chunk alpha
        alpha_raw = singles.tile([128, 2], F32, tag="alpha_raw")
        coefs = singles.tile([128, 12], F32, tag="coefs")       # scratch cols
        ab = singles.tile([128, 4], F32, tag="ab")  # cols: a k0,k1, b k0,k1

        # resident x shard: one [128, PIX] f32 tile per (b, k)
        xs = [[xpool.tile([128, PIX], F32, tag=f"xs{b}_{k}", name=f"xs{b}_{k}")
               for k in range(KC)] for b in range(B_LOC)]
        ws = {}

        # phase-C pools opened upfront: no SBUF/stack aliasing with the
        # phase-A scratch pools means no WAR deps delaying the border
        # memsets or the first conv matmuls
        phase_c_pools = ExitStack()
        xbpool = phase_c_pools.enter_context(tc.tile_pool(name="xbp", bufs=1))
        stpool = phase_c_pools.enter_context(tc.tile_pool(name="stage", bufs=8))
        cpsum = phase_c_pools.enter_context(
            tc.tile_pool(name="cpsum", bufs=6, space="PSUM"))

        if USE_FP8:
            xbp = [xbpool.tile([128, KC * PLANE_PAD], FP8, tag=f"xbp{b}",
                               name=f"xbp{b}")
                   for b in range(B_LOC)]
        else:
            xbp = [[xbpool.tile([128, XBP_LEN], BF16, tag=f"xbp{b}_{k}",
                                name=f"xbp{b}_{k}")
                    for k in range(KC)] for b in range(B_LOC)]

        # zero only the halo borders (the interior is fully overwritten
        # by the sign pass; the inter-plane alignment gap is never read)
        def memset_borders(t, base):
            nc.gpsimd.memset(t[:, base:base + 1], 0.0)           # lead elem
            nc.gpsimd.memset(t[:, base + 1:base + 1 + WP], 0.0)  # top row
            nc.gpsimd.memset(t[:, base + 1 + 57 * WP:base + 1 + 57 * WP + WP],
                             0.0)                                # bottom row
            side = (t[:, base + 1 + WP:base + 1 + 57 * WP]
                    .rearrange("p (h w) -> p h w", w=WP))
            nc.gpsimd.memset(side[:, :, 0:1], 0.0)               # left col
            nc.gpsimd.memset(side[:, :, WP - 1:WP], 0.0)         # right col
            nc.gpsimd.memset(t[:, base + 1 + PLANE:base + 1 + PLANE + 1], 0.0)

        for b in range(B_LOC):
            if USE_FP8:
                for k in range(KC):
                    memset_borders(xbp[b], k * PLANE_PAD)
            else:
                for k in range(KC):
                    memset_borders(xbp[b][k], 0)

        with (
            tc.tile_pool(name="wraw", bufs=1) as wraw_pool,
            tc.tile_pool(name="scr", bufs=3) as scr,
            tc.tile_pool(name="tpps", bufs=2, space="PSUM") as tp_psum,
        ):
            # ---- phase A: x shard DMA (first in the HWDGE queue — it
            # gates the sync-BN chain) + per-half-tile stats so the stat ops
            # trail the DMA stream by half a tile ----
            HPIX = PIX // 2
            for b in range(B_LOC):
                for k in range(KC):
                    for hf in range(2):
                        nc.sync.dma_start(
                            out=xs[b][k][:, hf * HPIX:(hf + 1) * HPIX],
                            in_=x[b, k * 128:(k + 1) * 128]
                            .rearrange("c h w -> c (h w)")[:, hf * HPIX:(hf + 1) * HPIX])
                        col = (k * B_LOC + b) * 2 + hf
                        xsl = xs[b][k][:, hf * HPIX:(hf + 1) * HPIX]
                        sa = scr.tile([128, HPIX], BF16, tag="scr_a", name="scr_a")
                        nc.scalar.activation(sa[:], xsl, AF.Copy,
                                             accum_out=psum_parts[:, col:col + 1])
                        sb = scr.tile([128, HPIX], BF16, tag="scr_b", name="scr_b")
                        nc.vector.scalar_tensor_tensor(
                            out=sb[:], in0=xsl, scalar=1.0, in1=xsl,
                            op0=mybir.AluOpType.mult, op1=mybir.AluOpType.mult,
                            accum_out=psq_parts[:, col:col + 1])
                        # HAM keep-warm: a discarded transpose gated on this
                        # half-tile's stat scratch paces PE activity through
                        # the DMA phase so the conv starts at the full clock
                        warm = tp_psum.tile([128, 128], BF16, tag="tp",
                                            name="warm")
                        nc.tensor.transpose(warm[:], sa[:, 0:128], identity[:])

            # one more keep-warm gated on the final stat scratch (~43us) to
            # narrow the PE-idle bridge before the weight transposes
            warm2 = tp_psum.tile([128, 128], BF16, tag="tp", name="warm2")
            nc.tensor.transpose(warm2[:], sb[:, 0:128], identity[:])

            # gamma/beta after the x stream so they don't delay it
            nc.sync.dma_start(out=gb[:, 0:2],
                              in_=gamma.rearrange("(k p) -> p k", p=128))
            nc.sync.dma_start(out=gb[:, 2:4],
                              in_=beta.rearrange("(k p) -> p k", p=128))

            # ---- finalize local stats + sync-BN all-reduce ----
            nc.vector.tensor_reduce(
                out=stats_local[:, 0:2],
                in_=psum_parts[:].rearrange("p (k bh) -> p k bh", k=KC),
                axis=mybir.AxisListType.X, op=add)
            nc.vector.tensor_reduce(
                out=stats_local[:, 2:4],
                in_=psq_parts[:].rearrange("p (k bh) -> p k bh", k=KC),
                axis=mybir.AxisListType.X, op=add)

            ccin = dram.tile([128, 4], F32, tag="ccin", name="ccin")
            ccout = dram.tile([128, 4], F32, tag="ccout", name="ccout")
            nc.sync.dma_start(out=ccin[:], in_=stats_local[:])
            if nc._use_cc:
                nc.gpsimd.collective_compute(
                    "AllReduce", add,
                    replica_groups=[list(range(nc._cc_devices))],
                    ins=[ccin.opt()], outs=[ccout.opt()])
            else:
                nc.sync.dma_start(out=ccout[:], in_=ccin[:])
            nc.sync.dma_start(out=gstats[:], in_=ccout[:])

            # ---- weight DMA + cheap weight math (fills engine idle while
            # the allreduce round-trips; w DMAs queue after x on HWDGE) ----
            wraws, wsigns = [], []
            for oc in range(2):
                wraw = wraw_pool.tile([128, C * 9], F32, tag=f"wraw{oc}",
                                      name=f"wraw{oc}")
                nc.sync.dma_start(
                    out=wraw[:],
                    in_=w[oc * 128:(oc + 1) * 128].rearrange("o c kh kw -> o (c kh kw)"))
                wsign = wraw_pool.tile([128, C * 9], BF16, tag=f"wsign{oc}",
                                       name=f"wsign{oc}")
                nc.scalar.activation(wsign[:], wraw[:], AF.Sign)
                wraws.append(wraw)
                wsigns.append(wsign)

            # ---- BN coefficients: a = gamma*inv, b = beta - mean*a ----
            mm = coefs[:, 0:4]       # mean k0,k1 | msq k0,k1
            mean = coefs[:, 0:2]
            msq = coefs[:, 2:4]
            m2 = coefs[:, 4:6]
            var = coefs[:, 6:8]
            rec = coefs[:, 8:10]
            inv = coefs[:, 10:12]
            # tiny [128,2] chain ops go on the otherwise-idle gpsimd queue so
            # they aren't latency-interleaved with bulk DVE work; only
            # reciprocal (DVE-only) and Sqrt (ACT) leave it
            nc.vector.tensor_scalar_mul(mm, gstats[:], 1.0 / N_TOTAL)
            nc.gpsimd.tensor_tensor(out=m2, in0=mean, in1=mean,
                                    op=mybir.AluOpType.mult)
            # var+eps = (msq + eps) - mean^2 in one op
            nc.vector.scalar_tensor_tensor(
                out=var, in0=msq, scalar=1e-5, in1=m2,
                op0=add, op1=mybir.AluOpType.subtract)
            nc.vector.reciprocal(rec, var)
            nc.scalar.activation(inv, rec, AF.Sqrt)
            nc.gpsimd.tensor_tensor(out=ab[:, 0:2], in0=gb[:, 0:2], in1=inv,
                                    op=mybir.AluOpType.mult)
            nc.gpsimd.tensor_tensor(out=ab[:, 2:4], in0=mean, in1=ab[:, 0:2],
                                    op=mybir.AluOpType.mult)
            nc.gpsimd.tensor_tensor(out=ab[:, 2:4], in0=gb[:, 2:4], in1=ab[:, 2:4],
                                    op=mybir.AluOpType.subtract)

            # alpha = mean|w| per output chunk (after the coef chain so these
            # big reduces don't block it in the DVE queue)
            for oc in range(2):
                nc.vector.tensor_reduce(
                    out=alpha_raw[:, oc:oc + 1], in_=wraws[oc][:],
                    axis=mybir.AxisListType.X, op=add, apply_absolute_value=True)
            nc.vector.tensor_scalar_mul(alpha[:], alpha_raw[:], 1.0 / (C * 9))

            # ---- weight transposes to [Cin, Cout] lhsT tiles (PE + DVE
            # copies; emitted after the coef chain so the small coef ops
            # aren't stuck behind 36 copies in the DVE queue) ----
            for oc in range(2):
                wsign3 = wsigns[oc][:].rearrange("o (kc t) -> o kc t", t=9)
                for tap in range(9):
                    if USE_FP8:
                        wt8 = wspool.tile([128, KC * 128], FP8,
                                          tag=f"ws8_{oc}_{tap}",
                                          name=f"ws8_{oc}_{tap}")
                        ws[(oc, tap)] = wt8
                        # both k-chunk transposes land in one PSUM tile so a
                        # single DVE copy drains them (DVE queue pressure)
                        pst = tp_psum.tile([128, KC * 128], BF16, tag="tp",
                                           name="tp")
                        for k in range(KC):
                            nc.tensor.transpose(
                                pst[:, k * 128:(k + 1) * 128],
                                wsign3[:, k * 128:(k + 1) * 128, tap],
                                identity[:])
                        nc.vector.tensor_copy(wt8[:], pst[:])
                    else:
                        for k in range(KC):
                            src = wsign3[:, k * 128:(k + 1) * 128, tap]
                            pst = tp_psum.tile([128, 128], BF16, tag="tp",
                                               name="tp")
                            nc.tensor.transpose(pst[:], src, identity[:])
                            wt = wspool.tile([128, 128], BF16,
                                             tag=f"ws{oc}_{k}_{tap}",
                                             name=f"ws{oc}_{k}_{tap}")
                            nc.vector.tensor_copy(wt[:], pst[:])
                            ws[(oc, k, tap)] = wt

        if nc._stage <= 1:
            # debug cutoff: dump coefs and bail
            nc.sync.dma_start(out=y[0, 0:128, 0, 0:4], in_=ab[:])
            phase_c_pools.close()
            return

        # ---- phase C: binarize into padded planes, then conv ----
        if True:
            def emit_sign(b, k, r0, r1):
                base = k * PLANE_PAD if USE_FP8 else 0
                tgt = xbp[b] if USE_FP8 else xbp[b][k]
                nr = r1 - r0
                lo = base + 1 + (1 + r0) * WP + 1
                interior = (tgt[:, lo:lo + (nr + 1) * WP]
                            .rearrange("p (h w) -> p h w", w=WP)[:, 0:nr, 0:W])
                nc.scalar.activation(
                    interior,
                    xs[b][k][:].rearrange("p (h w) -> p h w", w=W)[:, r0:r1, :],
                    AF.Sign,
                    bias=ab[:, 2 + k:3 + k], scale=ab[:, k:k + 1])

            # row splits, emitted in conv consumption order, so early conv
            # chunks unblock while later planes are still binarizing; the
            # first image is split finest since it gates the conv start
            for r0, r1 in ((0, 32), (32, H)):
                for k in range(KC):
                    emit_sign(0, k, r0, r1)
            for b in range(1, B_LOC):
                for k in range(KC):
                    emit_sign(b, k, 0, 32)
                for k in range(KC):
                    emit_sign(b, k, 32, H)

            if nc._stage <= 2:
                # debug cutoff: read back one row of every xbp plane
                dump = stpool.tile([128, 2 * B_LOC * W], F32, tag="dump",
                                   name="dump")
                for b in range(B_LOC):
                    for k in range(KC):
                        src = (xbp[b][:, k * PLANE_PAD + 60:k * PLANE_PAD + 60 + W]
                               if USE_FP8 else xbp[b][k][:, 60:60 + W])
                        nc.vector.tensor_copy(
                            dump[:, (b * KC + k) * W:(b * KC + k + 1) * W], src)
                nc.sync.dma_start(out=y[0, 0:128, 0:8, :],
                                  in_=dump[:].rearrange("p (r w) -> p r w", w=W))
                phase_c_pools.close()
                return

            for b in range(B_LOC):
                for h0 in range(0, H, R):
                    for oc in range(2):
                        acc = cpsum.tile([128, NF], F32, tag="acc", name="acc")
                        if USE_FP8:
                            xv = xbp[b][:].rearrange("p (i l) -> p i l",
                                                     l=PLANE_PAD)
                            for tap in range(9):
                                dh, dw = tap // 3, tap % 3
                                off = (h0 + dh) * WP + dw
                                lhsT = ws[(oc, tap)][:].rearrange(
                                    "p (i m) -> p i m", m=128)
                                nc.tensor.matmul(
                                    acc[:], lhsT, xv[:, :, off:off + NF],
                                    start=(tap == 0), stop=(tap == 8),
                                    perf_mode=mybir.MatmulPerfMode.DoubleRow)
                        else:
                            i = 0
                            for k in range(KC):
                                for dh in range(3):
                                    for dw in range(3):
                                        off = (h0 + dh) * WP + dw
                                        nc.tensor.matmul(
                                            acc[:],
                                            ws[(oc, k, dh * 3 + dw)][:],
                                            xbp[b][k][:, off:off + NF],
                                            start=(i == 0), stop=(i == 17))
                                        i += 1
                        stage = stpool.tile([128, R, W], F32, tag="stage",
                                            name="stage")
                        accv = (acc[:].rearrange("p (h w) -> p h w", w=WP)
                                [:, :, 1:1 + W])
                        nc.vector.tensor_scalar_mul(stage[:], accv,
                                                    alpha[:, oc:oc + 1])
                        nc.sync.dma_start(
                            out=y[b, oc * 128:(oc + 1) * 128, h0:h0 + R, :],
                            in_=stage[:])
        phase_c_pools.close()


def run_on_hw(x, weight, gamma, beta, **spmd_kwargs):
    nc = build_program()
    in_maps = []
    for i in range(N_CORES):
        in_maps.append({
            "x": np.ascontiguousarray(x[i * B_LOC:(i + 1) * B_LOC]),
            "weight": np.ascontiguousarray(weight),
            "gamma": np.ascontiguousarray(gamma),
            "beta": np.ascontiguousarray(beta),
        })
    return run_bass_kernel_spmd(nc, in_maps, core_ids=list(range(N_CORES)),
                                **spmd_kwargs)


def kernel(x: np.ndarray, weight: np.ndarray, gamma: np.ndarray,
           beta: np.ndarray) -> np.ndarray:
    # The first execution on a freshly-attached device occasionally reports
    # NRT_EXEC_UNIT_UNRECOVERABLE from residue of a prior process; an
    # immediate retry reliably succeeds.
    last_err = None
    for _ in range(3):
        try:
            res = run_on_hw(x, weight, gamma, beta)
            break
        except Exception as e:  # noqa: BLE001 - retry any transient runtime error
            last_err = e
    else:
        raise last_err
    out = np.concatenate([res.results[i]["y"] for i in range(N_CORES)], axis=0)
    return out.astype(np.float32)


if __name__ == "__main__":
    nc = build_program()
    print("build ok:", len(nc.inst_map), "instructions")

